# revision 39
# baseline (speedup 1.0000x reference)
"""GQA (16 q-heads / 4 KV groups, S=4096, D=1024, causal) on 8 TRN2 NeuronCores.

Sharding: tensor-parallel over query heads - 2 q-heads + their KV group per
core. wq/wk/wv column-sharded, wo row-sharded; the 8 partial outputs are
summed on the host (no device collectives needed).

Per-core program (bf16 matmuls, f32 PSUM):
  Projections (pipelined one chunk ahead of attention):
    set1 lhsT=[wq_h0|wq_h1] -> qT_sb [128, 512/chunk] (h0 parts 0:64, h1 64:128)
    set2 lhsT=[wk|wv]       -> kT duplicated to both partition halves of
                               kT2_sb [128, S]; vT -> vT_sb [64, S]
    v normal layout via PE-transpose of vT 128-col tiles -> vaug (ones col 64)
  Attention per q-chunk (512 q x 2 heads), per k-tile of 128 keys:
    qk: TWO row-tiled concurrent matmuls (tile_position (0,0)/(64,0)),
        h0/h1 -> separate PSUM banks of ps_s [128, 2, 512]
    exp(s/8) -> pt bf16 (one ACT instr over both banks)
    causal: diagonal strips compute only cols >= 128*r; tri-mask mul on the
        128-wide diagonal block per head
    ctx: 2 matmuls (lhsT=vaug [128 keys, 65], row 64 = ones -> denominators)
        accumulate ctx_ps [65, 2, 512]
  Normalize: DVE reciprocal of denom row (PSUM) -> bf16 -> DRAM ->
    partition-broadcast DMA -> rb [128, 1024]; ctx copied to stacked
    ctxT2 [128, S] (h1 via quadrant-shifted DVE write) and scaled in place.
  Out-proj: per 128-row block: 2 matmuls contraction 128 (both heads at once),
    psum -> bf16 ostage -> DMA to out partial [S, DIM] bf16.
Softmax uses no max-subtraction: s/8 ~ N(0,1) -> exp safe in f32.
"""

import numpy as np
import ml_dtypes

BF16 = ml_dtypes.bfloat16

S = 4096
DIN = 1024
DIM = 1024
NH, NKV, HD = 16, 4, 64
NCORES = 8
QC = 512          # q chunk width per head
NQC = S // QC     # 8
NKT = S // 128    # 32 k tiles

_CACHE = {}


def _build_nc(debug=False):
    import concourse.bass as bass
    import concourse.mybir as mybir
    import concourse.tile as tile
    from concourse import bacc
    from concourse.tile_rust import add_dep_helper
    from contextlib import ExitStack

    fp32 = mybir.dt.float32
    bf16 = mybir.dt.bfloat16
    Exp = mybir.ActivationFunctionType.Exp

    nc = bacc.Bacc()
    xT_d = nc.dram_tensor("xT", [DIN, S], bf16, kind="ExternalInput")
    # host pre-arranged [p, c, m]: row-contiguous DMA (128 descriptors)
    wqT_d = nc.dram_tensor("wqT", [128, 8 * 128], bf16, kind="ExternalInput")
    wkvT_d = nc.dram_tensor("wkvT", [128, 8 * 128], bf16, kind="ExternalInput")
    woT_d = nc.dram_tensor("woT", [128, DIM], bf16, kind="ExternalInput")
    mask_d = nc.dram_tensor("trimask", [128, 128], bf16, kind="ExternalInput")
    identf_d = nc.dram_tensor("identf", [64, 64], fp32, kind="ExternalInput")
    out_d = nc.dram_tensor("out", [S, DIM], bf16, kind="ExternalOutput")
    skind = {"kind": "ExternalOutput"} if debug else {}
    rec_d = nc.dram_tensor("recips_scratch", [NQC, 2 * QC], bf16, **skind)
    if debug:
        dbg_qT = nc.dram_tensor("dbg_qT", [128, NQC, QC], bf16, kind="ExternalOutput")
        dbg_kT = nc.dram_tensor("dbg_kT", [128, S], bf16, kind="ExternalOutput")
        dbg_vaug = nc.dram_tensor("dbg_vaug", [128, NKT, 128], bf16, kind="ExternalOutput")
        dbg_ctxT = nc.dram_tensor("dbg_ctxT", [128, S], bf16, kind="ExternalOutput")

    with ExitStack() as ctx:
        tc = ctx.enter_context(tile.TileContext(nc))
        singles = ctx.enter_context(tc.tile_pool(name="singles", bufs=1))
        pt_pool = ctx.enter_context(tc.tile_pool(name="pt", bufs=4))
        small = ctx.enter_context(tc.tile_pool(name="small", bufs=2))
        ostage = ctx.enter_context(tc.tile_pool(name="ostage", bufs=3))
        psum = ctx.enter_context(tc.tile_pool(name="psum", bufs=3, space="PSUM"))
        psum_ctx = ctx.enter_context(
            tc.tile_pool(name="psum_ctx", bufs=1, space="PSUM")
        )

        # ---- persistent SBUF tensors ----
        xT_sb = singles.tile([128, 8, S], bf16, tag="xT")
        wqT_sb = singles.tile([128, 8, 128], bf16, tag="wqT")
        wkvT_sb = singles.tile([128, 8, 128], bf16, tag="wkvT")
        woT_sb = singles.tile([128, DIM], bf16, tag="woT")
        mask_sb = singles.tile([128, 128], bf16, tag="mask")
        identf_sb = singles.tile([64, 64], fp32, tag="identf")
        ones_sb = singles.tile([1, 64], bf16, tag="ones")
        qT_sb = singles.tile([128, NQC, QC], bf16, tag="qT")
        kT2_sb = singles.tile([128, S], bf16, tag="kT2")
        vT_sb = singles.tile([64, S], fp32, tag="vT")
        vaug_sb = singles.tile([128, NKT, 128], bf16, tag="vaug")
        ctxT2_sb = singles.tile([128, S], bf16, tag="ctxT2")

        # ---- input DMAs ----
        nc.sync.dma_start(
            out=wqT_sb, in_=wqT_d[:].rearrange("p (c m) -> p c m", c=8)
        )
        nc.sync.dma_start(
            out=wkvT_sb, in_=wkvT_d[:].rearrange("p (c m) -> p c m", c=8)
        )
        nc.sync.dma_start(out=woT_sb, in_=woT_d[:])
        nc.sync.dma_start(out=mask_sb, in_=mask_d[:])
        nc.sync.dma_start(out=identf_sb, in_=identf_d[:])
        nc.vector.memset(ones_sb, 1.0)
        for n in range(NQC):
            for c in range(8):
                nc.sync.dma_start(
                    out=xT_sb[:, c, n * QC:(n + 1) * QC],
                    in_=xT_d[c * 128:(c + 1) * 128, n * QC:(n + 1) * QC],
                )
        nc.vector.memset(vaug_sb[:, :, 65:128], 0.0)
        nc.vector.memset(vaug_sb[:, :, 64:65], 1.0)

        def proj(n):
            """Chunk n projections: qT (both heads), kT (duplicated), vT."""
            ps = psum.tile([128, 2, QC], fp32, tag="ps_s")
            xs = xT_sb[:, :, n * QC:(n + 1) * QC]
            for c in range(8):
                nc.tensor.matmul(
                    ps[:, 0, :], wqT_sb[:, c, :], xs[:, c, :],
                    start=(c == 0), stop=(c == 7),
                )
                nc.tensor.matmul(
                    ps[:, 1, :], wkvT_sb[:, c, :], xs[:, c, :],
                    start=(c == 0), stop=(c == 7),
                )
            nc.vector.tensor_copy(vT_sb[:, n * QC:(n + 1) * QC], ps[64:128, 1, :])
            nc.vector.tensor_copy(kT2_sb[64:128, n * QC:(n + 1) * QC], ps[0:64, 1, :])
            nc.scalar.copy(kT2_sb[0:64, n * QC:(n + 1) * QC], ps[0:64, 1, :])
            nc.scalar.copy(qT_sb[:, n, :], ps[:, 0, :])
            for kt in range(4 * n, 4 * n + 4):
                tp = psum.tile([128, 2, QC], fp32, tag="ps_s")
                nc.tensor.transpose(
                    tp[:, 0, 0:64], vT_sb[:, kt * 128:(kt + 1) * 128], identf_sb
                )
                nc.vector.tensor_copy(vaug_sb[:, kt, 0:64], tp[:, 0, 0:64])

        def attention(qc):
            nkt = 4 * qc + 4
            ctx_ps = psum_ctx.tile([128, 2, QC], fp32, tag="ps_ctx")
            for kt in range(nkt):
                r = kt - 4 * qc
                off = 128 * r if r >= 1 else 0
                ps = psum.tile([128, 2, QC], fp32, tag="ps_s")
                pt = pt_pool.tile([128, 2, QC], bf16, tag="pt")
                ktl = kT2_sb[0:64, kt * 128:(kt + 1) * 128]
                kth = kT2_sb[64:128, kt * 128:(kt + 1) * 128]
                nc.tensor.matmul(
                    ps[:, 0, off:QC], ktl, qT_sb[0:64, qc, off:QC],
                    start=True, stop=True,
                )
                nc.tensor.matmul(
                    ps[:, 1, off:QC], kth, qT_sb[64:128, qc, off:QC],
                    start=True, stop=True, tile_position=(64, 0),
                )
                if r >= 0:  # strip holds the causal diagonal block
                    for h in range(2):
                        nc.scalar.activation(
                            pt[:, h, off:QC], ps[:, h, off:QC], Exp, scale=0.125
                        )
                        nc.gpsimd.tensor_mul(
                            pt[:, h, off:off + 128],
                            pt[:, h, off:off + 128],
                            mask_sb,
                        )
                else:
                    nc.scalar.activation(
                        pt[:, :, off:QC], ps[:, :, off:QC], Exp, scale=0.125
                    )
                va = vaug_sb[:, kt, :]
                nc.tensor.matmul(
                    ctx_ps[:, 0, off:QC], va, pt[:, 0, off:QC],
                    start=(kt == 0), stop=(kt == nkt - 1),
                )
                nc.tensor.matmul(
                    ctx_ps[:, 1, off:QC], va, pt[:, 1, off:QC],
                    start=(kt == 0), stop=(kt == nkt - 1),
                )
            return ctx_ps

        def finish_ctx(qc, ctx_ps, last=False):
            """Copy raw ctx to stacked sbuf + launch reciprocal broadcast."""
            cs = slice(qc * QC, (qc + 1) * QC)
            nc.vector.tensor_copy(ctxT2_sb[0:64, cs], ctx_ps[0:64, 0, :])
            nc.vector.tensor_copy(ctxT2_sb[64:128, cs], ctx_ps[0:64, 1, :])
            rec = small.tile([1, 2, QC], fp32, tag="rec")
            nc.vector.tensor_copy(rec, ctx_ps[64:65, :, :])
            nc.vector.reciprocal_approx_fast(rec, rec)
            recb = small.tile([1, 2 * QC], bf16, tag="recb")
            nc.vector.tensor_copy(recb[:, 0:QC], rec[:, 0, :])
            nc.vector.tensor_copy(recb[:, QC:2 * QC], rec[:, 1, :])
            if last:
                return recb
            w1 = nc.sync.dma_start(out=rec_d[qc:qc + 1, :], in_=recb)
            rb = small.tile([128, 2 * QC], bf16, tag="rb")
            src = rec_d[qc, :]
            r1 = nc.sync.dma_start(
                out=rb,
                in_=bass.AP(tensor=src.tensor, offset=src.offset,
                            ap=[[0, 128]] + list(src.ap)),
            )
            add_dep_helper(r1.ins, w1.ins, reason="recips dram RAW")
            return rb

        def normalize_last(qc, recb):
            """PE-broadcast the reciprocals; skip the DRAM round trip."""
            bps = psum.tile([128, 2, QC], fp32, tag="ps_s")
            nc.tensor.matmul(
                bps[0:64, 0, :], ones_sb, recb[0:1, 0:QC],
                start=True, stop=True,
            )
            nc.tensor.matmul(
                bps[0:64, 1, :], ones_sb, recb[0:1, QC:2 * QC],
                start=True, stop=True,
            )
            rbs = small.tile([128, 2, QC], bf16, tag="rbs")
            nc.vector.tensor_copy(rbs[0:64, 0, :], bps[0:64, 0, :])
            nc.vector.tensor_copy(rbs[64:128, 1, :], bps[0:64, 1, :])
            cs = slice(qc * QC, (qc + 1) * QC)
            nc.vector.tensor_mul(
                ctxT2_sb[0:64, cs], ctxT2_sb[0:64, cs], rbs[0:64, 0, :]
            )
            nc.vector.tensor_mul(
                ctxT2_sb[64:128, cs], ctxT2_sb[64:128, cs], rbs[64:128, 1, :]
            )

        def normalize(qc, rb):
            cs = slice(qc * QC, (qc + 1) * QC)
            nc.gpsimd.tensor_mul(
                ctxT2_sb[0:64, cs], ctxT2_sb[0:64, cs], rb[0:64, 0:QC]
            )
            nc.gpsimd.tensor_mul(
                ctxT2_sb[64:128, cs], ctxT2_sb[64:128, cs], rb[64:128, QC:2 * QC]
            )

        def outproj(qc):
            for rc in range(4 * qc, 4 * qc + 4):
                ps_o = psum.tile([128, 2, QC], fp32, tag="ps_s")
                lh = ctxT2_sb[:, rc * 128:(rc + 1) * 128]
                for e in range(2):
                    nc.tensor.matmul(
                        ps_o[:, e, :], lh, woT_sb[:, e * QC:(e + 1) * QC],
                        start=True, stop=True,
                    )
                ot = ostage.tile([128, DIM], bf16, tag="ot")
                nc.scalar.copy(ot[:, 0:QC], ps_o[:, 0, :])
                nc.vector.tensor_copy(ot[:, QC:DIM], ps_o[:, 1, :])
                nc.sync.dma_start(
                    out=out_d[rc * 128:(rc + 1) * 128, :], in_=ot
                )

        # ---- main pipeline (proj + v-transposes run two chunks ahead) ----
        proj(0)
        proj(1)
        prev = None  # (qc, rb) awaiting normalize+outproj
        for qc in range(NQC):
            if prev is not None:
                normalize(prev[0], prev[1])
            ctx_ps = attention(qc)
            rb = finish_ctx(qc, ctx_ps, last=(qc == NQC - 1))
            if qc + 2 < NQC:
                proj(qc + 2)
            if prev is not None:
                outproj(prev[0])
            prev = (qc, rb)
        normalize_last(prev[0], prev[1])
        outproj(prev[0])

        if debug:
            nc.sync.dma_start(out=dbg_qT[:], in_=qT_sb)
            nc.sync.dma_start(out=dbg_kT[:], in_=kT2_sb)
            nc.sync.dma_start(out=dbg_vaug[:], in_=vaug_sb)
            nc.sync.dma_start(out=dbg_ctxT[:], in_=ctxT2_sb)

    nc.compile()
    return nc


def _get_nc():
    if "nc" not in _CACHE:
        _CACHE["nc"] = _build_nc()
    return _CACHE["nc"]


def _prep_inputs(x, wq, wk, wv, wo):
    GS = NH // NKV
    x2 = np.asarray(x, np.float32).reshape(S, DIN)
    xT = np.ascontiguousarray(x2.T).astype(BF16)
    tri = (np.arange(128)[None, :] >= np.arange(128)[:, None]).astype(BF16)
    in_maps = []
    for c in range(NCORES):
        h0 = 2 * c
        g = h0 // GS
        wq_c = np.asarray(wq, np.float32)[h0 * HD:(h0 + 2) * HD, :]
        wkv_c = np.concatenate(
            [
                np.asarray(wk, np.float32)[g * HD:(g + 1) * HD, :],
                np.asarray(wv, np.float32)[g * HD:(g + 1) * HD, :],
            ],
            axis=0,
        )
        woT_c = np.asarray(wo, np.float32)[:, h0 * HD:(h0 + 2) * HD].T

        def prearrange(wT):  # [1024, 128] -> [p, c*m] = [128, 1024]
            return np.ascontiguousarray(
                wT.reshape(8, 128, 128).transpose(1, 0, 2).reshape(128, 1024)
            )

        in_maps.append(
            {
                "xT": xT,
                "wqT": prearrange(np.ascontiguousarray(wq_c.T)).astype(BF16),
                "wkvT": prearrange(np.ascontiguousarray(wkv_c.T)).astype(BF16),
                "woT": np.ascontiguousarray(woT_c).astype(BF16),
                "trimask": tri,
                "identf": np.eye(64, dtype=np.float32),
            }
        )
    return in_maps


def _run(in_maps, trace=False):
    import sys
    if "/opt/trn_rl_repo" not in sys.path:
        sys.path.insert(0, "/opt/trn_rl_repo")
    from concourse.bass_utils import run_bass_kernel_spmd

    nc = _get_nc()
    res = run_bass_kernel_spmd(nc, in_maps, list(range(NCORES)), trace=trace)
    return res


def kernel(x, wq, wk, wv, wo):
    in_maps = _prep_inputs(x, wq, wk, wv, wo)
    res = _run(in_maps)
    parts = np.stack(
        [np.asarray(r["out"], np.float32) for r in res.results]
    )
    out = parts.sum(axis=0, dtype=np.float64).astype(np.float32)
    return out.reshape(1, S, DIM)


# revision 46
# speedup vs baseline: 1.1384x; 1.1384x over previous
"""GQA (16 q-heads / 4 KV groups, S=4096, D=1024, causal) on 8 TRN2 NeuronCores.

Sharding: tensor-parallel over query heads - 2 q-heads + their KV group per
core. wq/wk/wv column-sharded, wo row-sharded; the 8 partial outputs are
summed on the host (no device collectives needed).

Per-core program (bf16 matmuls, f32 PSUM):
  Projections (pipelined one chunk ahead of attention):
    set1 lhsT=[wq_h0|wq_h1] -> qT_sb [128, 512/chunk] (h0 parts 0:64, h1 64:128)
    set2 lhsT=[wk|wv]       -> kT duplicated to both partition halves of
                               kT2_sb [128, S]; vT -> vT_sb [64, S]
    v normal layout via PE-transpose of vT 128-col tiles -> vaug (ones col 64)
  Attention per q-chunk (512 q x 2 heads), per k-tile of 128 keys:
    qk: TWO row-tiled concurrent matmuls (tile_position (0,0)/(64,0)),
        h0/h1 -> separate PSUM banks of ps_s [128, 2, 512]
    exp(s/8) -> pt bf16 (one ACT instr over both banks)
    causal: diagonal strips compute only cols >= 128*r; tri-mask mul on the
        128-wide diagonal block per head
    ctx: 2 matmuls (lhsT=vaug [128 keys, 65], row 64 = ones -> denominators)
        accumulate ctx_ps [65, 2, 512]
  Normalize: DVE reciprocal of denom row (PSUM) -> bf16 -> DRAM ->
    partition-broadcast DMA -> rb [128, 1024]; ctx copied to stacked
    ctxT2 [128, S] (h1 via quadrant-shifted DVE write) and scaled in place.
  Out-proj: per 128-row block: 2 matmuls contraction 128 (both heads at once),
    psum -> bf16 ostage -> DMA to out partial [S, DIM] bf16.
Softmax uses no max-subtraction: s/8 ~ N(0,1) -> exp safe in f32.
"""

import numpy as np
import ml_dtypes

BF16 = ml_dtypes.bfloat16

S = 4096
DIN = 1024
DIM = 1024
NH, NKV, HD = 16, 4, 64
NCORES = 8
QC = 512          # q chunk width per head
NQC = S // QC     # 8
NKT = S // 128    # 32 k tiles

_CACHE = {}


def _build_nc(debug=False):
    import concourse.bass as bass
    import concourse.mybir as mybir
    import concourse.tile as tile
    from concourse import bacc
    from concourse.tile_rust import add_dep_helper
    from contextlib import ExitStack

    fp32 = mybir.dt.float32
    bf16 = mybir.dt.bfloat16
    Exp = mybir.ActivationFunctionType.Exp

    nc = bacc.Bacc()
    xT_d = nc.dram_tensor("xT", [DIN, S], bf16, kind="ExternalInput")
    # host pre-arranged [p, c, m]: row-contiguous DMA (128 descriptors)
    wqT_d = nc.dram_tensor("wqT", [128, 8 * 128], bf16, kind="ExternalInput")
    wkvT_d = nc.dram_tensor("wkvT", [128, 8 * 128], bf16, kind="ExternalInput")
    woT_d = nc.dram_tensor("woT", [128, DIM], bf16, kind="ExternalInput")
    mask_d = nc.dram_tensor("trimask", [128, 128], bf16, kind="ExternalInput")
    out_d = nc.dram_tensor("out", [S, DIM], bf16, kind="ExternalOutput")
    skind = {"kind": "ExternalOutput"} if debug else {}
    rec_d = nc.dram_tensor("recips_scratch", [NQC, 2 * QC], bf16, **skind)
    if debug:
        dbg_qT = nc.dram_tensor("dbg_qT", [128, NQC, QC], bf16, kind="ExternalOutput")
        dbg_kT = nc.dram_tensor("dbg_kT", [128, S], bf16, kind="ExternalOutput")
        dbg_vaug = nc.dram_tensor("dbg_vaug", [128, NKT, 128], bf16, kind="ExternalOutput")
        dbg_ctxT = nc.dram_tensor("dbg_ctxT", [128, S], bf16, kind="ExternalOutput")

    with ExitStack() as ctx:
        tc = ctx.enter_context(tile.TileContext(nc))
        singles = ctx.enter_context(tc.tile_pool(name="singles", bufs=1))
        pt_pool = ctx.enter_context(tc.tile_pool(name="pt", bufs=4))
        small = ctx.enter_context(tc.tile_pool(name="small", bufs=2))
        ostage = ctx.enter_context(tc.tile_pool(name="ostage", bufs=3))
        psum = ctx.enter_context(tc.tile_pool(name="psum", bufs=3, space="PSUM"))
        psum_ctx = ctx.enter_context(
            tc.tile_pool(name="psum_ctx", bufs=1, space="PSUM")
        )

        # ---- persistent SBUF tensors ----
        xT_sb = singles.tile([128, 8, S], bf16, tag="xT")
        wqT_sb = singles.tile([128, 8, 128], bf16, tag="wqT")
        wkvT_sb = singles.tile([128, 8, 128], bf16, tag="wkvT")
        woT_sb = singles.tile([128, DIM], bf16, tag="woT")
        mask_sb = singles.tile([128, 128], bf16, tag="mask")
        ones_sb = singles.tile([1, 64], bf16, tag="ones")
        qT_sb = singles.tile([128, NQC, QC], bf16, tag="qT")
        kT2_sb = singles.tile([128, S], bf16, tag="kT2")
        vT_sb = singles.tile([64, S], bf16, tag="vT")
        vaug_sb = singles.tile([128, NKT, 128], bf16, tag="vaug")
        ctxT2_sb = singles.tile([128, S], bf16, tag="ctxT2")

        # ---- input DMAs ----
        nc.sync.dma_start(
            out=wqT_sb, in_=wqT_d[:].rearrange("p (c m) -> p c m", c=8)
        )
        nc.sync.dma_start(
            out=wkvT_sb, in_=wkvT_d[:].rearrange("p (c m) -> p c m", c=8)
        )
        nc.sync.dma_start(out=woT_sb, in_=woT_d[:])
        nc.sync.dma_start(out=mask_sb, in_=mask_d[:])
        nc.vector.memset(ones_sb, 1.0)

        def load_x(ns):
            for n in ns:
                for c in range(8):
                    nc.sync.dma_start(
                        out=xT_sb[:, c, n * QC:(n + 1) * QC],
                        in_=xT_d[c * 128:(c + 1) * 128, n * QC:(n + 1) * QC],
                    )

        load_x([0, 1])
        nc.vector.memset(vaug_sb[:, :, 65:128], 0.0)
        nc.vector.memset(vaug_sb[:, :, 64:65], 1.0)

        def proj(n):
            """Chunk n projections: qT (both heads), kT (duplicated), vT."""
            ps = psum.tile([128, 2, QC], fp32, tag="ps_s")
            xs = xT_sb[:, :, n * QC:(n + 1) * QC]
            for c in range(8):
                nc.tensor.matmul(
                    ps[:, 0, :], wqT_sb[:, c, :], xs[:, c, :],
                    start=(c == 0), stop=(c == 7),
                )
                nc.tensor.matmul(
                    ps[:, 1, :], wkvT_sb[:, c, :], xs[:, c, :],
                    start=(c == 0), stop=(c == 7),
                )
            nc.vector.tensor_copy(vT_sb[:, n * QC:(n + 1) * QC], ps[64:128, 1, :])
            nc.vector.tensor_copy(kT2_sb[64:128, n * QC:(n + 1) * QC], ps[0:64, 1, :])
            nc.scalar.copy(kT2_sb[0:64, n * QC:(n + 1) * QC], ps[0:64, 1, :])
            nc.scalar.copy(qT_sb[:, n, :], ps[:, 0, :])
            for kt in range(4 * n, 4 * n + 4):
                nc.sync.dma_start_transpose(
                    out=vaug_sb[:, kt, 0:64],
                    in_=vT_sb[:, kt * 128:(kt + 1) * 128],
                )

        def attention(qc):
            nkt = 4 * qc + 4
            ctx_ps = psum_ctx.tile([128, 2, QC], fp32, tag="ps_ctx")
            for kt in range(nkt):
                r = kt - 4 * qc
                off = 128 * r if r >= 1 else 0
                ps = psum.tile([128, 2, QC], fp32, tag="ps_s")
                pt = pt_pool.tile([128, 2, QC], bf16, tag="pt")
                ktl = kT2_sb[0:64, kt * 128:(kt + 1) * 128]
                kth = kT2_sb[64:128, kt * 128:(kt + 1) * 128]
                nc.tensor.matmul(
                    ps[:, 0, off:QC], ktl, qT_sb[0:64, qc, off:QC],
                    start=True, stop=True,
                )
                nc.tensor.matmul(
                    ps[:, 1, off:QC], kth, qT_sb[64:128, qc, off:QC],
                    start=True, stop=True, tile_position=(64, 0),
                )
                if r >= 0:  # strip holds the causal diagonal block
                    for h in range(2):
                        nc.scalar.activation(
                            pt[:, h, off:QC], ps[:, h, off:QC], Exp, scale=0.125
                        )
                        nc.gpsimd.tensor_mul(
                            pt[:, h, off:off + 128],
                            pt[:, h, off:off + 128],
                            mask_sb,
                        )
                else:
                    nc.scalar.activation(
                        pt[:, :, off:QC], ps[:, :, off:QC], Exp, scale=0.125
                    )
                va = vaug_sb[:, kt, :]
                nc.tensor.matmul(
                    ctx_ps[:, 0, off:QC], va, pt[:, 0, off:QC],
                    start=(kt == 0), stop=(kt == nkt - 1),
                )
                nc.tensor.matmul(
                    ctx_ps[:, 1, off:QC], va, pt[:, 1, off:QC],
                    start=(kt == 0), stop=(kt == nkt - 1),
                )
            return ctx_ps

        def finish_ctx(qc, ctx_ps, last=False):
            """Copy raw ctx to stacked sbuf + launch reciprocal broadcast."""
            cs = slice(qc * QC, (qc + 1) * QC)
            nc.vector.tensor_copy(ctxT2_sb[0:64, cs], ctx_ps[0:64, 0, :])
            nc.vector.tensor_copy(ctxT2_sb[64:128, cs], ctx_ps[0:64, 1, :])
            rec = small.tile([1, 2, QC], fp32, tag="rec")
            nc.vector.tensor_copy(rec, ctx_ps[64:65, :, :])
            nc.vector.reciprocal_approx_fast(rec, rec)
            recb = small.tile([1, 2 * QC], bf16, tag="recb")
            nc.vector.tensor_copy(recb[:, 0:QC], rec[:, 0, :])
            nc.vector.tensor_copy(recb[:, QC:2 * QC], rec[:, 1, :])
            if last:
                return recb
            w1 = nc.sync.dma_start(out=rec_d[qc:qc + 1, :], in_=recb)
            rb = small.tile([128, 2 * QC], bf16, tag="rb")
            src = rec_d[qc, :]
            r1 = nc.sync.dma_start(
                out=rb,
                in_=bass.AP(tensor=src.tensor, offset=src.offset,
                            ap=[[0, 128]] + list(src.ap)),
            )
            add_dep_helper(r1.ins, w1.ins, reason="recips dram RAW")
            return rb

        def normalize_last(qc, recb):
            """PE-broadcast the reciprocals; skip the DRAM round trip."""
            bps = psum.tile([128, 2, QC], fp32, tag="ps_s")
            nc.tensor.matmul(
                bps[0:64, 0, :], ones_sb, recb[0:1, 0:QC],
                start=True, stop=True,
            )
            nc.tensor.matmul(
                bps[0:64, 1, :], ones_sb, recb[0:1, QC:2 * QC],
                start=True, stop=True,
            )
            rbs = small.tile([128, 2, QC], bf16, tag="rbs")
            nc.vector.tensor_copy(rbs[0:64, 0, :], bps[0:64, 0, :])
            nc.vector.tensor_copy(rbs[64:128, 1, :], bps[0:64, 1, :])
            cs = slice(qc * QC, (qc + 1) * QC)
            nc.vector.tensor_mul(
                ctxT2_sb[0:64, cs], ctxT2_sb[0:64, cs], rbs[0:64, 0, :]
            )
            nc.vector.tensor_mul(
                ctxT2_sb[64:128, cs], ctxT2_sb[64:128, cs], rbs[64:128, 1, :]
            )

        def normalize(qc, rb):
            cs = slice(qc * QC, (qc + 1) * QC)
            nc.gpsimd.tensor_mul(
                ctxT2_sb[0:64, cs], ctxT2_sb[0:64, cs], rb[0:64, 0:QC]
            )
            nc.gpsimd.tensor_mul(
                ctxT2_sb[64:128, cs], ctxT2_sb[64:128, cs], rb[64:128, QC:2 * QC]
            )

        def outproj(qc):
            for rc in range(4 * qc, 4 * qc + 4):
                ps_o = psum.tile([128, 2, QC], fp32, tag="ps_s")
                lh = ctxT2_sb[:, rc * 128:(rc + 1) * 128]
                for e in range(2):
                    nc.tensor.matmul(
                        ps_o[:, e, :], lh, woT_sb[:, e * QC:(e + 1) * QC],
                        start=True, stop=True,
                    )
                ot = ostage.tile([128, DIM], bf16, tag="ot")
                nc.scalar.copy(ot[:, 0:QC], ps_o[:, 0, :])
                nc.vector.tensor_copy(ot[:, QC:DIM], ps_o[:, 1, :])
                nc.sync.dma_start(
                    out=out_d[rc * 128:(rc + 1) * 128, :], in_=ot
                )

        # ---- main pipeline (proj + v-transposes run two chunks ahead) ----
        proj(0)
        proj(1)
        load_x(range(2, NQC))
        prev = None  # (qc, rb) awaiting normalize+outproj
        for qc in range(NQC):
            if prev is not None:
                normalize(prev[0], prev[1])
            ctx_ps = attention(qc)
            rb = finish_ctx(qc, ctx_ps, last=(qc == NQC - 1))
            if prev is not None:
                outproj(prev[0])
            if qc + 2 < NQC:
                proj(qc + 2)
            prev = (qc, rb)
        normalize_last(prev[0], prev[1])
        outproj(prev[0])

        if debug:
            nc.sync.dma_start(out=dbg_qT[:], in_=qT_sb)
            nc.sync.dma_start(out=dbg_kT[:], in_=kT2_sb)
            nc.sync.dma_start(out=dbg_vaug[:], in_=vaug_sb)
            nc.sync.dma_start(out=dbg_ctxT[:], in_=ctxT2_sb)

    nc.compile()
    return nc


def _get_nc():
    if "nc" not in _CACHE:
        _CACHE["nc"] = _build_nc()
    return _CACHE["nc"]


def _prep_inputs(x, wq, wk, wv, wo):
    GS = NH // NKV
    x2 = np.asarray(x, np.float32).reshape(S, DIN)
    xT = np.ascontiguousarray(x2.T).astype(BF16)
    tri = (np.arange(128)[None, :] >= np.arange(128)[:, None]).astype(BF16)
    in_maps = []
    for c in range(NCORES):
        h0 = 2 * c
        g = h0 // GS
        wq_c = np.asarray(wq, np.float32)[h0 * HD:(h0 + 2) * HD, :]
        wkv_c = np.concatenate(
            [
                np.asarray(wk, np.float32)[g * HD:(g + 1) * HD, :],
                np.asarray(wv, np.float32)[g * HD:(g + 1) * HD, :],
            ],
            axis=0,
        )
        woT_c = np.asarray(wo, np.float32)[:, h0 * HD:(h0 + 2) * HD].T

        def prearrange(wT):  # [1024, 128] -> [p, c*m] = [128, 1024]
            return np.ascontiguousarray(
                wT.reshape(8, 128, 128).transpose(1, 0, 2).reshape(128, 1024)
            )

        in_maps.append(
            {
                "xT": xT,
                "wqT": prearrange(np.ascontiguousarray(wq_c.T)).astype(BF16),
                "wkvT": prearrange(np.ascontiguousarray(wkv_c.T)).astype(BF16),
                "woT": np.ascontiguousarray(woT_c).astype(BF16),
                "trimask": tri,
            }
        )
    return in_maps


def _run(in_maps, trace=False):
    import sys
    if "/opt/trn_rl_repo" not in sys.path:
        sys.path.insert(0, "/opt/trn_rl_repo")
    from concourse.bass_utils import run_bass_kernel_spmd

    nc = _get_nc()
    res = run_bass_kernel_spmd(nc, in_maps, list(range(NCORES)), trace=trace)
    return res


def kernel(x, wq, wk, wv, wo):
    in_maps = _prep_inputs(x, wq, wk, wv, wo)
    res = _run(in_maps)
    parts = np.stack(
        [np.asarray(r["out"], np.float32) for r in res.results]
    )
    out = parts.sum(axis=0, dtype=np.float64).astype(np.float32)
    return out.reshape(1, S, DIM)
